# revision 1
# baseline (speedup 1.0000x reference)
"""Trainium2 Bass kernel for nn_CAModel (neural cellular automaton step).

Strategy (pure data parallel, B=32 -> 4 images per core x 8 cores):
- Host pre-transposes to channel-major padded layout; device partition p =
  (img_local, half, channel) = 4*2*16 = 128.  All spatial shifts become
  free-dim offsets (row pitch 130, zero ring).
- Depthwise sobel conv as separable shifted adds on VectorE in bf16.
- fc0 as 3 accumulating K=32 matmuls per group (zero-padded weights per
  group parity), 4 partition strips run concurrently on the PE sub-arrays.
- relu PSUM->SBUF copy split between ScalarE and VectorE, bf16 out.
- fc1 as K=128 -> M=32 matmul pairs accumulating both group parities.
- residual + update mask + alive mask (3x3 maxpool in a strip layout,
  scatter/broadcast via SBUF-SBUF DMA) on VectorE.
"""

import dataclasses
import numpy as np
import ml_dtypes

import concourse.bass as bass
import concourse.tile as tile
from concourse import mybir, bass_utils
import bass_rust

F32 = mybir.dt.float32
BF16 = mybir.dt.bfloat16
ALU = mybir.AluOpType
ACTF = mybir.ActivationFunctionType

N_CORES = 8
B, H, W, C = 32, 128, 128, 16
HID = 128
IMGS = B // N_CORES          # 4 images per core
GRP = IMGS * 2               # 8 (img, half) groups per core
PW = W + 2                   # padded row pitch 130
PR = H // 2 + 2              # padded rows per half 66
NPAD = PR * PW               # 8580
NPIX = (H // 2) * W          # 8192 interior pixels per group
CHUNK = 1024                 # pixels per MLP chunk (8 interior rows)
NCHUNK = NPIX // CHUNK       # 8
X2G = 128                    # guard elems around x2 free dim
RELU_PATTERN = (True, True, False)  # True -> ScalarE


def _split_multiwaits(nc):
    """walrus in this env only supports one sem-wait per instruction."""
    n = 0
    for f in nc.m.functions:
        for bb in f.blocks:
            out = []
            changed = False
            for inst in bb.instructions:
                si = inst.sync_info
                if si is not None and len(si.on_wait) > 1:
                    waits = list(si.on_wait)
                    for k, w in enumerate(waits[:-1]):
                        nop = mybir.InstNoOp(
                            name=f"{inst.name}_ws{k}",
                            sync_info=mybir.SyncInfo(on_wait=[w], on_update=[]),
                            bass_nofuse=True,
                            engine=inst.engine,
                        )
                        nc.register_instruction(nop, overwrite=True)
                        out.append(nop)
                        n += 1
                    inst.sync_info = mybir.SyncInfo(
                        on_wait=[waits[-1]], on_update=list(si.on_update)
                    )
                    changed = True
                out.append(inst)
            if changed:
                bb.instructions[:] = out
    return n


def _mk_ap(ap, offset, dims):
    return dataclasses.replace(ap, offset=offset, ap=[list(d) for d in dims])


def build_program():
    nc = bass.Bass()

    xpad_d = nc.dram_tensor("xpad", [128, NPAD], F32, kind="ExternalInput").ap()
    u16_d = nc.dram_tensor("u16", [128, NPIX], BF16, kind="ExternalInput").ap()
    astrip_d = nc.dram_tensor("astrip", [128, 780], F32, kind="ExternalInput").ap()
    w0_d = {}
    for feat in ("id", "dx", "dy"):
        for gg in range(2):
            w0_d[(feat, gg)] = nc.dram_tensor(
                f"w0{feat}{gg}", [128, 128], BF16, kind="ExternalInput"
            ).ap()
    w1_d = [
        nc.dram_tensor(f"w1{gg}", [128, 32], BF16, kind="ExternalInput").ap()
        for gg in range(2)
    ]
    sel_d = nc.dram_tensor("sel", [128, 2048], BF16, kind="ExternalInput").ap()
    out_d = nc.dram_tensor("out", [128, NPIX], F32, kind="ExternalOutput").ap()

    with tile.TileContext(nc) as tc:
        with tc.tile_pool(name="persist", bufs=1) as pp:
            # --- persistent tiles ---
            xpad = pp.tile([128, NPAD], F32, tag="xpad")
            xb = pp.tile([128, NPAD + 4], BF16, tag="xb")        # data at +2
            ydx = pp.tile([128, 64 * PW], BF16, tag="ydx")
            ydy = pp.tile([128, 64 * PW], BF16, tag="ydy")
            astrip = pp.tile([128, 780], F32, tag="astrip")
            a2strip = pp.tile([128, 780], F32, tag="a2strip")
            selt = pp.tile([128, 2048], BF16, tag="selt")
            nc.sync.dma_start(out=selt[:, :], in_=sel_d)
            w0t = {k: pp.tile([128, 128], BF16, tag=f"w0{k[0]}{k[1]}", name=f"w0t{k[0]}{k[1]}") for k in w0_d}
            w1t = [pp.tile([128, 32], BF16, tag=f"w1{gg}", name=f"w1t{gg}") for gg in range(2)]

            # --- input DMAs ---
            for k in w0_d:
                nc.sync.dma_start(out=w0t[k][:, :], in_=w0_d[k])
            for gg in range(2):
                nc.sync.dma_start(out=w1t[gg][:, :], in_=w1_d[gg])
            CAST_BANDS = [(0, 18), (18, 34), (34, 50), (50, 66)]
            for lo, hi in CAST_BANDS:
                nc.sync.dma_start(
                    out=xpad[:, lo * PW : hi * PW], in_=xpad_d[:, lo * PW : hi * PW]
                )
            nc.sync.dma_start(out=astrip[:, :], in_=astrip_d)

            # --- cast to bf16 (ScalarE, per band) ---
            for lo, hi in CAST_BANDS:
                nc.scalar.activation(
                    out=xb[:, 2 + lo * PW : 2 + hi * PW],
                    in_=xpad[:, lo * PW : hi * PW],
                    func=ACTF.Copy,
                )

            # --- conv (VectorE, bf16) ---
            with tc.tile_pool(name="convp", bufs=1) as cp:
                xx2 = cp.tile([128, NPAD], BF16, tag="xx2")
                t_a = cp.tile([128, NPAD + 4], BF16, tag="c1", name="sv_t")

                tv = cp.tile([128, NPAD + 4], BF16, tag="c2", name="tv_t")
                t_b = cp.tile([128, NPAD + 4], BF16, tag="c1b", name="sh_t")
                th = cp.tile([128, NPAD + 4], BF16, tag="c2b", name="th_t")
                # per band: xx2/sh over padded-row range [lo,hi);
                # sv/tv/ydx/ydy over interior rows [max(lo,1) .. min(hi,65))
                for bi, (lo, hi) in enumerate(CAST_BANDS):
                    nc.vector.tensor_scalar_mul(
                        xx2[:, lo * PW : hi * PW], xb[:, 2 + lo * PW : 2 + hi * PW], 2.0
                    )
                    nc.vector.tensor_tensor(
                        out=t_b[:, 2 + lo * PW : 2 + hi * PW],
                        in0=xb[:, 1 + lo * PW : 1 + hi * PW],
                        in1=xb[:, 3 + lo * PW : 3 + hi * PW],
                        op=ALU.add,
                    )
                    nc.vector.tensor_tensor(
                        out=th[:, 2 + lo * PW : 2 + hi * PW],
                        in0=t_b[:, 2 + lo * PW : 2 + hi * PW],
                        in1=xx2[:, lo * PW : hi * PW],
                        op=ALU.add,
                    )
                    if bi == 0:
                        continue
                    # dx/dy outputs for rows covered by casts emitted so far
                    lo, hi = CAST_BANDS[bi - 1]
                    rl, rh_ = max(lo, 1), min(hi, 65)
                    nc.vector.tensor_tensor(
                        out=t_a[:, 2 + rl * PW : 2 + rh_ * PW],
                        in0=xb[:, 2 + (rl - 1) * PW : 2 + (rh_ - 1) * PW],
                        in1=xb[:, 2 + (rl + 1) * PW : 2 + (rh_ + 1) * PW],
                        op=ALU.add,
                    )
                    nc.vector.tensor_tensor(
                        out=tv[:, 2 + rl * PW : 2 + rh_ * PW],
                        in0=t_a[:, 2 + rl * PW : 2 + rh_ * PW],
                        in1=xx2[:, rl * PW : rh_ * PW],
                        op=ALU.add,
                    )
                    nc.vector.tensor_tensor(
                        out=ydx[:, (rl - 1) * PW : (rh_ - 1) * PW],
                        in0=tv[:, 3 + rl * PW : 3 + rh_ * PW],
                        in1=tv[:, 1 + rl * PW : 1 + rh_ * PW],
                        op=ALU.subtract,
                    )
                    nc.vector.tensor_tensor(
                        out=ydy[:, (rl - 1) * PW : (rh_ - 1) * PW],
                        in0=th[:, 2 + (rl + 1) * PW : 2 + (rh_ + 1) * PW],
                        in1=th[:, 2 + (rl - 1) * PW : 2 + (rh_ - 1) * PW],
                        op=ALU.subtract,
                    )
                for lo, hi in CAST_BANDS[-1:]:
                    rl, rh_ = max(lo, 1), min(hi, 65)
                    nc.vector.tensor_tensor(
                        out=t_a[:, 2 + rl * PW : 2 + rh_ * PW],
                        in0=xb[:, 2 + (rl - 1) * PW : 2 + (rh_ - 1) * PW],
                        in1=xb[:, 2 + (rl + 1) * PW : 2 + (rh_ + 1) * PW],
                        op=ALU.add,
                    )
                    nc.vector.tensor_tensor(
                        out=tv[:, 2 + rl * PW : 2 + rh_ * PW],
                        in0=t_a[:, 2 + rl * PW : 2 + rh_ * PW],
                        in1=xx2[:, rl * PW : rh_ * PW],
                        op=ALU.add,
                    )
                    nc.vector.tensor_tensor(
                        out=ydx[:, (rl - 1) * PW : (rh_ - 1) * PW],
                        in0=tv[:, 3 + rl * PW : 3 + rh_ * PW],
                        in1=tv[:, 1 + rl * PW : 1 + rh_ * PW],
                        op=ALU.subtract,
                    )
                    nc.vector.tensor_tensor(
                        out=ydy[:, (rl - 1) * PW : (rh_ - 1) * PW],
                        in0=th[:, 2 + (rl + 1) * PW : 2 + (rh_ + 1) * PW],
                        in1=th[:, 2 + (rl - 1) * PW : 2 + (rh_ - 1) * PW],
                        op=ALU.subtract,
                    )

            # --- MLP + residual ---
            xbr = xb[:, 2 : 2 + NPAD].rearrange("p (r w) -> p r w", w=PW)
            ydxr = ydx[:, :].rearrange("p (r w) -> p r w", w=PW)
            ydyr = ydy[:, :].rearrange("p (r w) -> p r w", w=PW)
            xintr = xpad[:, :].rearrange("p (r w) -> p r w", w=PW)

            relu_i = 0
            lp_cm = tc.tile_pool(name="late", bufs=1)
            lp = lp_cm.__enter__()
            x2 = lp.tile([128, NPIX + 2 * X2G], F32, tag="x2")   # data at +X2G
            nc.vector.memset(x2[:, 0:X2G], 0.0)
            nc.vector.memset(x2[:, X2G + NPIX : NPIX + 2 * X2G], 0.0)
            u16 = lp.tile([128, NPIX], BF16, tag="ul", name="u16")
            nc.sync.dma_start(out=u16[:, :], in_=u16_d)
            x2r = x2[:, X2G : X2G + NPIX].rearrange("p (r w) -> p r w", w=W)
            with (
                tc.tile_pool(name="mlp", bufs=1) as mp,
                tc.tile_pool(name="psum", bufs=1, space="PSUM") as psp,
            ):
                prepool = pp.tile([128, 512], F32, tag="prepool")

                def emit_prepool():
                    vm_e = pp.tile([128, 524], F32, tag="vm_e")
                    t1_e = pp.tile([128, 524], F32, tag="t1_e")
                    t2_e = pp.tile([128, 524], F32, tag="t2_e")
                    nc.vector.tensor_tensor(
                        out=t1_e[:, 0:520], in0=astrip[:, 0:520],
                        in1=astrip[:, 130:650], op=ALU.max,
                    )
                    nc.vector.tensor_tensor(
                        out=vm_e[:, 0:520], in0=t1_e[:, 0:520],
                        in1=astrip[:, 260:780], op=ALU.max,
                    )
                    nc.vector.tensor_tensor(
                        out=t2_e[:, 0:519], in0=vm_e[:, 0:519], in1=vm_e[:, 1:520],
                        op=ALU.max,
                    )
                    _vmr = vm_e[:, 0:520].rearrange("p (r w) -> p r w", w=130)
                    _t2r = t2_e[:, 0:520].rearrange("p (r w) -> p r w", w=130)
                    _ppr = prepool[:, :].rearrange("p (r w) -> p r w", w=128)
                    nc.vector.tensor_tensor(
                        out=_ppr[:, 0:4, :], in0=_t2r[:, 0:4, 0:128],
                        in1=_vmr[:, 0:4, 2:130], op=ALU.max,
                    )

                for k in range(NCHUNK):
                    if k == 3:
                        emit_prepool()
                    r0 = 8 * k  # interior row base of chunk
                    dxp = psp.tile([128, CHUNK], F32, tag="dxp", bufs=2)
                    for j in range(4):
                        for gg in range(2):
                            hp = psp.tile([128, CHUNK], F32, tag="hp", bufs=2)
                            for sub in range(2):
                                rr = r0 + 4 * sub
                                hps = hp[:, sub * 512 : sub * 512 + 512]
                                rhss = [
                                    xbr[32 * j : 32 * j + 32, 1 + rr : 5 + rr, 1:129],
                                    ydxr[32 * j : 32 * j + 32, rr : rr + 4, 1:129],
                                    ydyr[32 * j : 32 * j + 32, rr : rr + 4, 1:129],
                                ]
                                for fi, feat in enumerate(("id", "dx", "dy")):
                                    nc.tensor.matmul(
                                        hps,
                                        w0t[(feat, gg)][32 * j : 32 * j + 32, :],
                                        rhss[fi],
                                        start=(fi == 0),
                                        stop=(fi == 2),
                                        tile_position=(32 * j, 0),
                                    )
                            rh = mp.tile([128, CHUNK], BF16, tag="rh", bufs=4)
                            if RELU_PATTERN[relu_i % len(RELU_PATTERN)]:
                                nc.scalar.activation(
                                    out=rh[:, :], in_=hp[:, :], func=ACTF.Relu
                                )
                            else:
                                nc.vector.tensor_scalar_max(rh[:, :], hp[:, :], 0.0)
                            relu_i += 1
                            for sub in range(2):
                                nc.tensor.matmul(
                                    dxp[32 * j : 32 * j + 32, sub * 512 : sub * 512 + 512],
                                    w1t[gg][:, :],
                                    rh[:, sub * 512 : sub * 512 + 512],
                                    start=(gg == 0),
                                    stop=(gg == 1),
                                    tile_position=(0, 32 * j),
                                )
                    st = lp.tile([128, 2048], F32, tag="st", name=f"st{k}", bufs=2)
                    nc.vector.tensor_tensor(
                        out=st[:, 0:CHUNK],
                        in0=dxp[:, :],
                        in1=u16[:, k * CHUNK : (k + 1) * CHUNK],
                        op=ALU.mult,
                    )
                    str_ = st[:, 0:CHUNK].rearrange("p (r w) -> p r w", w=W)
                    nc.vector.tensor_tensor(
                        out=x2r[:, r0 : r0 + 8, :],
                        in0=xintr[:, 1 + r0 : 9 + r0, 1:129],
                        in1=str_,
                        op=ALU.add,
                    )

            # --- alive masks ---
            alp = lp
            if True:
                nc.vector.memset(a2strip[:, :], 0.0)
                # scatter x2 alpha into strip layout: one DMA per halo row r
                PITCH = NPIX + 2 * X2G
                x2ap = x2[:, :]
                a2ap = a2strip[:, :]
                for r in range(6):
                    src = _mk_ap(
                        x2ap, 3 * PITCH + 128 * r,
                        [[16 * PITCH, 8], [512, 16], [1, 128]],
                    )
                    dst = _mk_ap(a2ap, 130 * r + 1, [[780, 128], [1, 128]])
                    nc.sync.dma_start(out=dst, in_=src)
                # cross-half halo rows
                nc.sync.dma_start(
                    out=_mk_ap(a2ap, 15 * 780 + 5 * 130 + 1, [[32 * 780, 4], [1, 128]]),
                    in_=_mk_ap(x2ap, 19 * PITCH + X2G, [[32 * PITCH, 4], [1, 128]]),
                )
                nc.sync.dma_start(
                    out=_mk_ap(a2ap, 16 * 780 + 1, [[32 * 780, 4], [1, 128]]),
                    in_=_mk_ap(
                        x2ap, 3 * PITCH + X2G + 63 * 128, [[32 * PITCH, 4], [1, 128]]
                    ),
                )

                def pool3(src_t, dst_t):
                    vm = alp.tile([128, 524], F32, tag="vm")
                    t1 = alp.tile([128, 524], F32, tag="t1")
                    nc.vector.tensor_tensor(
                        out=t1[:, 0:520], in0=src_t[:, 0:520], in1=src_t[:, 130:650],
                        op=ALU.max,
                    )
                    nc.vector.tensor_tensor(
                        out=vm[:, 0:520], in0=t1[:, 0:520], in1=src_t[:, 260:780],
                        op=ALU.max,
                    )
                    t2 = alp.tile([128, 524], F32, tag="t2")
                    nc.vector.tensor_tensor(
                        out=t2[:, 0:519], in0=vm[:, 0:519], in1=vm[:, 1:520],
                        op=ALU.max,
                    )
                    vmr = vm[:, 0:520].rearrange("p (r w) -> p r w", w=130)
                    t2r = t2[:, 0:520].rearrange("p (r w) -> p r w", w=130)
                    dstr = dst_t[:, :].rearrange("p (r w) -> p r w", w=128)
                    nc.vector.tensor_tensor(
                        out=dstr[:, 0:4, :],
                        in0=t2r[:, 0:4, 0:128],
                        in1=vmr[:, 0:4, 2:130],
                        op=ALU.max,
                    )

                postpool = alp.tile([128, 512], F32, tag="postpool")
                pool3(a2strip, postpool)
                pmin = alp.tile([128, 512], F32, tag="pmin")
                nc.vector.tensor_tensor(
                    out=pmin[:, :], in0=prepool[:, :], in1=postpool[:, :], op=ALU.min
                )
                lifes = alp.tile([128, 512], BF16, tag="lifes")
                nc.vector.tensor_scalar(
                    out=lifes[:, :], in0=pmin[:, :], scalar1=0.1, scalar2=None,
                    op0=ALU.is_gt,
                )

            # --- final mask multiply + store ---
            with tc.tile_pool(name="psum2", bufs=1, space="PSUM") as psp2:
                for k in range(4):
                    lps = psp2.tile([128, 2048], F32, tag="lps", name=f"lps{k}", bufs=2)
                    for tl in range(4):
                        t = 4 * k + tl
                        nc.tensor.matmul(
                            lps[:, 512 * tl : 512 * tl + 512],
                            selt[:, 128 * t : 128 * t + 128],
                            lifes[:, 0:512],
                            start=True,
                            stop=True,
                        )
                    ot = lp.tile([128, 2048], F32, tag="st", name=f"ot{k}", bufs=2)
                    nc.vector.tensor_tensor(
                        out=ot[:, :],
                        in0=x2[:, X2G + 2048 * k : X2G + 2048 * (k + 1)],
                        in1=lps[:, :],
                        op=ALU.mult,
                    )
                    eng = nc.sync if k % 2 == 0 else nc.scalar
                    eng.dma_start(
                        out=out_d[:, 2048 * k : 2048 * (k + 1)], in_=ot[:, :]
                    )

            lp_cm.__exit__(None, None, None)

    _split_multiwaits(nc)
    return nc


def host_prep(x, w0, w1, rand_mask):
    bf = ml_dtypes.bfloat16
    xt = np.ascontiguousarray(x.transpose(0, 3, 1, 2))  # [B, C, H, W]

    xp = np.zeros((B, 2, C, PR, PW), np.float32)
    xp[:, 0, :, 1:66, 1:129] = xt[:, :, 0:65, :]
    xp[:, 1, :, 0:65, 1:129] = xt[:, :, 63:128, :]
    xp = xp.reshape(B, 2, C, NPAD)

    u = (rand_mask[..., 0] <= 0.5).astype(np.float32).reshape(B, 2, 64, W)
    u16 = np.ascontiguousarray(
        np.broadcast_to(u[:, :, None], (B, 2, C, 64, W))
    ).astype(bf).reshape(B, 2, C, NPIX)

    apad = np.zeros((B, H + 2, PW), np.float32)
    apad[:, 1:129, 1:129] = x[..., 3]
    idx = 4 * np.arange(32)[:, None] + np.arange(6)[None, :]
    astr = apad[:, idx, :].reshape(B, 32, 780)  # [B, strip, 6*130]

    W0id = w0[:, 0::3]
    W0dx = w0[:, 1::3] / 8.0
    W0dy = w0[:, 2::3] / 8.0
    w0_arrs = {}
    for feat, Wm in (("id", W0id), ("dx", W0dx), ("dy", W0dy)):
        blk = Wm.T.astype(bf)  # [16 c, 128 o]
        for gg in range(2):
            t = np.zeros((128, 128), bf)
            for j in range(4):
                t[32 * j + 16 * gg : 32 * j + 16 * gg + 16, :] = blk
            w0_arrs[(feat, gg)] = t
    w1_arrs = []
    for gg in range(2):
        t = np.zeros((128, 32), bf)
        t[:, 16 * gg : 16 * gg + 16] = w1.T.astype(bf)
        w1_arrs.append(t)

    sel = np.zeros((128, 2048), bf)
    for t in range(16):
        for p in range(128):
            g = p // 16
            sel[16 * g + t, 128 * t + p] = 1.0

    in_maps = []
    for ci in range(N_CORES):
        sl = slice(IMGS * ci, IMGS * (ci + 1))
        m = {
            "xpad": np.ascontiguousarray(xp[sl]).reshape(128, NPAD),
            "u16": np.ascontiguousarray(u16[sl]).reshape(128, NPIX),
            "astrip": np.ascontiguousarray(astr[sl]).reshape(128, 780),
            "sel": sel,
            "w10": w1_arrs[0],
            "w11": w1_arrs[1],
        }
        for (feat, gg), arr in w0_arrs.items():
            m[f"w0{feat}{gg}"] = arr
        in_maps.append(m)
    return in_maps


def host_post(results):
    out = np.empty((B, H, W, C), np.float32)
    for ci in range(N_CORES):
        o = results[ci]["out"].reshape(IMGS, 2, C, 64, W)
        out[IMGS * ci : IMGS * (ci + 1)] = o.transpose(0, 1, 3, 4, 2).reshape(
            IMGS, H, W, C
        )
    return out


_CACHE = {}


def kernel(x, w0, w1, rand_mask, _trace=False):
    x = np.asarray(x, np.float32)
    w0 = np.asarray(w0, np.float32)
    w1 = np.asarray(w1, np.float32)
    rand_mask = np.asarray(rand_mask, np.float32)

    if "nc" not in _CACHE:
        _CACHE["nc"] = build_program()
    nc = _CACHE["nc"]

    in_maps = host_prep(x, w0, w1, rand_mask)
    res = bass_utils.run_bass_kernel_spmd(
        nc, in_maps, core_ids=list(range(N_CORES)), trace=_trace
    )
    _CACHE["last_result"] = res
    return host_post(res.results)



# revision 13
# speedup vs baseline: 1.2304x; 1.2304x over previous
"""Trainium2 Bass kernel for nn_CAModel (neural cellular automaton step).

Strategy (pure data parallel, B=32 -> 4 images per core x 8 cores):
- Host pre-transposes to channel-major padded layout; device partition p =
  (img_local, half, channel) = 4*2*16 = 128.  All spatial shifts become
  free-dim offsets (row pitch 130, zero ring).
- Depthwise sobel conv as separable shifted adds on VectorE in bf16,
  band-local temps (8 interior rows per band), fused (2x + t) via
  scalar_tensor_tensor.
- Conv outputs DMA-repacked into 48-feature K-strips: y48 tiles with
  partitions 64*sg + (16*feat + chan), sg = img//2 super-group, free dim =
  (quad, row, col) pixel stream.  fc0 is then ONE K=48 matmul per
  512-px window per group (3x fewer PE rows than feature-accumulation).
- fc1: K=128 M=32 matmuls with zero-padded w1lo/w1hi writing the two
  16-partition halves of each 32-strip (accumulated pair, junk-free).
- Window-ordered epilogue: per 512-px window the 8 group matmuls fill all
  128 PSUM partitions -> full-width residual + fire-mask ops.
- relu PSUM->SBUF split across ScalarE/GpSimd/VectorE; alive-mask maxpool
  in strip layout; life broadcast via PE sel-matmul.
"""

import dataclasses
import numpy as np
import ml_dtypes

import concourse.bass as bass
import concourse.tile as tile
from concourse import mybir, bass_utils

F32 = mybir.dt.float32
BF16 = mybir.dt.bfloat16
ALU = mybir.AluOpType
ACTF = mybir.ActivationFunctionType

N_CORES = 8
B, H, W, C = 32, 128, 128, 16
HID = 128
IMGS = B // N_CORES          # 4 images per core
PW = W + 2                   # padded row pitch 130
PR = H // 2 + 2              # padded rows per half 66
NPAD = PR * PW               # 8580
NPIX = (H // 2) * W          # 8192 interior pixels per group
X2G = 128                    # guard elems around x2 free dim
PITCH = NPIX + 2 * X2G
NB = 8                       # conv bands, 8 interior rows each
NW = 16                      # MLP windows, 512 px each
RELU_PATTERN = ("s",) * 18 + ("v",)


def _split_multiwaits(nc):
    """walrus in this env only supports one sem-wait per instruction."""
    n = 0
    for f in nc.m.functions:
        for bb in f.blocks:
            out = []
            changed = False
            for inst in bb.instructions:
                si = inst.sync_info
                if si is not None and len(si.on_wait) > 1:
                    waits = list(si.on_wait)
                    for k, w in enumerate(waits[:-1]):
                        nop = mybir.InstNoOp(
                            name=f"{inst.name}_ws{k}",
                            sync_info=mybir.SyncInfo(on_wait=[w], on_update=[]),
                            bass_nofuse=True,
                            engine=inst.engine,
                        )
                        nc.register_instruction(nop, overwrite=True)
                        out.append(nop)
                        n += 1
                    inst.sync_info = mybir.SyncInfo(
                        on_wait=[waits[-1]], on_update=list(si.on_update)
                    )
                    changed = True
                out.append(inst)
            if changed:
                bb.instructions[:] = out
    return n


def _mk_ap(ap, offset, dims):
    return dataclasses.replace(ap, offset=offset, ap=[list(d) for d in dims])


def build_program():
    nc = bass.Bass()

    xpad_d = nc.dram_tensor("xpad", [128, NPAD], F32, kind="ExternalInput").ap()
    u16_d = nc.dram_tensor("u16", [128, NPIX], BF16, kind="ExternalInput").ap()
    astrip_d = nc.dram_tensor("astrip", [128, 780], F32, kind="ExternalInput").ap()
    w048_d = nc.dram_tensor("w048", [128, 128], BF16, kind="ExternalInput").ap()
    w1lo_d = nc.dram_tensor("w1lo", [128, 32], BF16, kind="ExternalInput").ap()
    w1hi_d = nc.dram_tensor("w1hi", [128, 32], BF16, kind="ExternalInput").ap()
    sel_d = nc.dram_tensor("sel", [128, 2048], BF16, kind="ExternalInput").ap()
    out_d = nc.dram_tensor("out", [128, NPIX], F32, kind="ExternalOutput").ap()

    with tile.TileContext(nc) as tc:
        with tc.tile_pool(name="persist", bufs=1) as pp:
            xpad = pp.tile([128, NPAD], F32, tag="xpad")
            y48 = [
                pp.tile([128, 4 * 2048], BF16, tag=f"y48_{t}", name=f"y48_{t}")
                for t in range(4)
            ]
            u16 = pp.tile([128, NPIX], BF16, tag="u16")
            astrip = pp.tile([128, 780], F32, tag="astrip")
            a2strip = pp.tile([128, 780], F32, tag="a2strip")
            selt = pp.tile([128, 2048], BF16, tag="selt")
            w048t = pp.tile([128, 128], BF16, tag="w048")
            w1t = [pp.tile([128, 32], BF16, tag=f"w1_{i}", name=f"w1_{i}") for i in range(2)]
            prepool = pp.tile([128, 512], F32, tag="prepool")
            postpool = pp.tile([128, 512], F32, tag="postpool")
            pmin = pp.tile([128, 512], F32, tag="pmin")
            lifes = pp.tile([128, 512], BF16, tag="lifes")

            # --- input DMAs ---
            nc.sync.dma_start(out=w048t[:, :], in_=w048_d)
            nc.sync.dma_start(out=w1t[0][:, :], in_=w1lo_d)
            nc.sync.dma_start(out=w1t[1][:, :], in_=w1hi_d)
            CAST_BANDS = [(0, 18), (18, 34), (34, 50), (50, 66)]
            for lo, hi in CAST_BANDS:
                nc.sync.dma_start(
                    out=xpad[:, lo * PW : hi * PW], in_=xpad_d[:, lo * PW : hi * PW]
                )
            nc.scalar.dma_start(out=u16[:, :], in_=u16_d)
            nc.scalar.dma_start(out=astrip[:, :], in_=astrip_d)
            nc.scalar.dma_start(out=selt[:, :], in_=sel_d)
            nc.vector.memset(a2strip[:, :], 0.0)

            def pool3(src_t, dst_t, eng):
                """3x3 maxpool in the 6-row strip layout (4 out rows/strip)."""
                vm = pp.tile([128, 524], F32, tag="vm")
                t1 = pp.tile([128, 524], F32, tag="t1")
                t2 = pp.tile([128, 524], F32, tag="t2")
                eng.tensor_tensor(
                    out=t1[:, 0:520], in0=src_t[:, 0:520], in1=src_t[:, 130:650],
                    op=ALU.max,
                )
                eng.tensor_tensor(
                    out=vm[:, 0:520], in0=t1[:, 0:520], in1=src_t[:, 260:780],
                    op=ALU.max,
                )
                eng.tensor_tensor(
                    out=t2[:, 0:519], in0=vm[:, 0:519], in1=vm[:, 1:520],
                    op=ALU.max,
                )
                vmr = vm[:, 0:520].rearrange("p (r w) -> p r w", w=130)
                t2r = t2[:, 0:520].rearrange("p (r w) -> p r w", w=130)
                dstr = dst_t[:, :].rearrange("p (r w) -> p r w", w=128)
                eng.tensor_tensor(
                    out=dstr[:, 0:4, :], in0=t2r[:, 0:4, 0:128],
                    in1=vmr[:, 0:4, 2:130], op=ALU.max,
                )

            # --- conv (band-local temps) + repack DMAs ---
            with tc.tile_pool(name="convp", bufs=1) as cp:
                xb = cp.tile([128, NPAD], BF16, tag="xb")
                for lo, hi in CAST_BANDS:
                    nc.scalar.activation(
                        out=xb[:, lo * PW : hi * PW],
                        in_=xpad[:, lo * PW : hi * PW],
                        func=ACTF.Copy,
                    )
                xb3 = xb[:, :].rearrange("p (r w) -> p r w", w=PW)
                dma_engs = (nc.sync, nc.scalar, nc.gpsimd)
                dma_i = 0
                for b in range(NB):
                    r0 = 8 * b  # temp base padded row; interior rows r0+1..r0+9
                    tb = cp.tile([128, 10 * PW], BF16, tag="tb")
                    th = cp.tile([128, 10 * PW], BF16, tag="th")
                    ta = cp.tile([128, 8 * PW], BF16, tag="ta")
                    tv = cp.tile([128, 8 * PW], BF16, tag="tv")
                    ydxb = cp.tile([128, 8 * PW], BF16, tag="ydx", bufs=2)
                    ydyb = cp.tile([128, 8 * PW], BF16, tag="ydy", bufs=2)
                    tb3 = tb[:, :].rearrange("p (r w) -> p r w", w=PW)
                    th3 = th[:, :].rearrange("p (r w) -> p r w", w=PW)
                    ta3 = ta[:, :].rearrange("p (r w) -> p r w", w=PW)
                    tv3 = tv[:, :].rearrange("p (r w) -> p r w", w=PW)
                    ydx3 = ydxb[:, :].rearrange("p (r w) -> p r w", w=PW)
                    ydy3 = ydyb[:, :].rearrange("p (r w) -> p r w", w=PW)
                    nc.vector.tensor_tensor(
                        out=tb3[:, :, 1:129], in0=xb3[:, r0 : r0 + 10, 0:128],
                        in1=xb3[:, r0 : r0 + 10, 2:130], op=ALU.add,
                    )
                    nc.vector.scalar_tensor_tensor(
                        out=th3[:, :, 1:129], in0=xb3[:, r0 : r0 + 10, 1:129],
                        scalar=2.0, in1=tb3[:, :, 1:129],
                        op0=ALU.mult, op1=ALU.add,
                    )
                    nc.vector.tensor_tensor(
                        out=ta3[:, :, :], in0=xb3[:, r0 : r0 + 8, :],
                        in1=xb3[:, r0 + 2 : r0 + 10, :], op=ALU.add,
                    )
                    nc.vector.scalar_tensor_tensor(
                        out=tv3[:, :, :], in0=xb3[:, r0 + 1 : r0 + 9, :],
                        scalar=2.0, in1=ta3[:, :, :],
                        op0=ALU.mult, op1=ALU.add,
                    )
                    nc.vector.tensor_tensor(
                        out=ydx3[:, :, 1:129], in0=tv3[:, :, 2:130],
                        in1=tv3[:, :, 0:128], op=ALU.subtract,
                    )
                    nc.vector.tensor_tensor(
                        out=ydy3[:, :, 1:129], in0=th3[:, 2:10, 1:129],
                        in1=th3[:, 0:8, 1:129], op=ALU.subtract,
                    )
                    # repack: one DMA per (sg, feat) covering 4 quads x 16 chan
                    t = b // 2
                    lr0 = 8 * b - 16 * t
                    ydst = y48[t][:, :]
                    for sg in range(2):
                        srcs = [
                            (0, xb[:, :], NPAD, (1 + 8 * b) * PW + 1),
                            (1, ydxb[:, :], 8 * PW, 1),
                            (2, ydyb[:, :], 8 * PW, 1),
                        ]
                        for ft, src_ap, spitch, soff in srcs:
                            for q in range(4):
                                dst = _mk_ap(
                                    ydst,
                                    (64 * sg + 16 * ft) * 8192 + q * 2048 + lr0 * 128,
                                    [[8192, 16], [128, 8], [1, 128]],
                                )
                                src = _mk_ap(
                                    src_ap,
                                    (64 * sg + 16 * q) * spitch + soff,
                                    [[spitch, 16], [PW, 8], [1, 128]],
                                )
                                dma_engs[dma_i % 3].dma_start(out=dst, in_=src)
                                dma_i += 1

            # pre-life maxpool while PE spins up
            pool3(astrip, prepool, nc.vector)

            # --- MLP + residual, window-ordered, lag-2 software pipeline ---
            lp_cm = tc.tile_pool(name="late", bufs=1)
            lp = lp_cm.__enter__()
            x2 = lp.tile([128, NPIX + 2 * X2G], F32, tag="x2")
            nc.vector.memset(x2[:, 0:X2G], 0.0)
            nc.vector.memset(x2[:, X2G + NPIX : NPIX + 2 * X2G], 0.0)
            x2r = x2[:, X2G : X2G + NPIX].rearrange("p (r w) -> p r w", w=W)
            xintr = xpad[:, :].rearrange("p (r w) -> p r w", w=PW)

            with (
                tc.tile_pool(name="mlp", bufs=1) as mp,
                tc.tile_pool(name="psum", bufs=1, space="PSUM") as psp,
            ):
                stages = [(w, g) for w in range(NW) for g in range(8)]
                hp_t = {}
                rh_t = {}
                dxp_t = {}
                relu_i = 0

                def emit_fc0(i):
                    w, g = stages[i]
                    sg, q = g // 4, g % 4
                    t, lw = w // 4, w % 4
                    hp = psp.tile([128, 512], F32, tag="hp", bufs=4)
                    hp_t[i] = hp
                    nc.tensor.matmul(
                        hp[:, :],
                        w048t[64 * sg : 64 * sg + 48, :],
                        y48[t][64 * sg : 64 * sg + 48, q * 2048 + lw * 512 : q * 2048 + lw * 512 + 512],
                        start=True, stop=True,
                        tile_position=(64 * sg, 0),
                    )

                def emit_relu(i):
                    nonlocal relu_i
                    rh = mp.tile([128, 512], BF16, tag="rh", bufs=4)
                    rh_t[i] = rh
                    kind = RELU_PATTERN[relu_i % len(RELU_PATTERN)]
                    relu_i += 1
                    hp = hp_t.pop(i)
                    if kind == "s":
                        nc.scalar.activation(out=rh[:, :], in_=hp[:, :], func=ACTF.Relu)
                    else:
                        nc.vector.tensor_scalar_max(rh[:, :], hp[:, :], 0.0)

                def emit_fc1(i):
                    w, g = stages[i]
                    a = g // 2
                    if g == 0:
                        dxp_t[w] = psp.tile(
                            [128, 512], F32, tag="dxp", bufs=2, name="dxp"
                        )
                    dxp = dxp_t[w]
                    rh = rh_t.pop(i)
                    nc.tensor.matmul(
                        dxp[32 * a : 32 * a + 32, :],
                        w1t[g % 2][:, :],
                        rh[:, :],
                        start=(g % 2 == 0), stop=(g % 2 == 1),
                        tile_position=(0, 32 * a),
                    )

                def emit_epilogue(w):
                    # mask-mult drains dx PSUM->SBUF on Vector (gpsimd and
                    # DMA can't read PSUM), residual add on GpSimd
                    dxp = dxp_t.pop(w)
                    st = lp.tile([128, 512], F32, tag="st", bufs=2)
                    nc.vector.tensor_tensor(
                        out=st[:, :], in0=dxp[:, :],
                        in1=u16[:, w * 512 : (w + 1) * 512], op=ALU.mult,
                    )
                    st3 = st[:, :].rearrange("p (r w) -> p r w", w=W)
                    nc.vector.tensor_tensor(
                        out=x2r[:, 4 * w : 4 * w + 4, :],
                        in0=xintr[:, 1 + 4 * w : 5 + 4 * w, 1:129],
                        in1=st3,
                        op=ALU.add,
                    )

                LAG = 2
                for i in range(len(stages) + LAG):
                    if i < len(stages):
                        emit_fc0(i)
                    j = i - LAG
                    if j >= 0:
                        emit_relu(j)
                        emit_fc1(j)
                        wj, gj = stages[j]
                        if gj == 7:
                            emit_epilogue(wj)

            # --- alive masks ---
            x2ap = x2[:, :]
            a2ap = a2strip[:, :]
            for r in range(6):
                src = _mk_ap(
                    x2ap, 3 * PITCH + 128 * r,
                    [[16 * PITCH, 8], [512, 16], [1, 128]],
                )
                dst = _mk_ap(a2ap, 130 * r + 1, [[780, 128], [1, 128]])
                eng = nc.sync if r % 2 == 0 else nc.scalar
                eng.dma_start(out=dst, in_=src)
            nc.sync.dma_start(
                out=_mk_ap(a2ap, 15 * 780 + 5 * 130 + 1, [[32 * 780, 4], [1, 128]]),
                in_=_mk_ap(x2ap, 19 * PITCH + X2G, [[32 * PITCH, 4], [1, 128]]),
            )
            nc.scalar.dma_start(
                out=_mk_ap(a2ap, 16 * 780 + 1, [[32 * 780, 4], [1, 128]]),
                in_=_mk_ap(
                    x2ap, 3 * PITCH + X2G + 63 * 128, [[32 * PITCH, 4], [1, 128]]
                ),
            )
            pool3(a2strip, postpool, nc.vector)
            nc.vector.tensor_tensor(
                out=pmin[:, :], in0=prepool[:, :], in1=postpool[:, :], op=ALU.min
            )
            nc.vector.tensor_scalar(
                out=lifes[:, :], in0=pmin[:, :], scalar1=0.1, scalar2=None,
                op0=ALU.is_gt,
            )

            # --- final mask multiply + store ---
            with tc.tile_pool(name="psum2", bufs=1, space="PSUM") as psp2:
                for k in range(4):
                    lps = psp2.tile([128, 2048], F32, tag="lps", name=f"lps{k}", bufs=2)
                    for tl in range(4):
                        t = 4 * k + tl
                        nc.tensor.matmul(
                            lps[:, 512 * tl : 512 * tl + 512],
                            selt[:, 128 * t : 128 * t + 128],
                            lifes[:, 0:512],
                            start=True,
                            stop=True,
                        )
                    ot = lp.tile([128, 2048], F32, tag="ot", name=f"ot{k}", bufs=2)
                    eng = nc.vector
                    eng.tensor_tensor(
                        out=ot[:, :],
                        in0=x2[:, X2G + 2048 * k : X2G + 2048 * (k + 1)],
                        in1=lps[:, :],
                        op=ALU.mult,
                    )
                    deng = nc.sync if k % 2 == 0 else nc.scalar
                    deng.dma_start(
                        out=out_d[:, 2048 * k : 2048 * (k + 1)], in_=ot[:, :]
                    )

            lp_cm.__exit__(None, None, None)

    _split_multiwaits(nc)
    return nc


def host_prep(x, w0, w1, rand_mask):
    bf = ml_dtypes.bfloat16
    xt = np.ascontiguousarray(x.transpose(0, 3, 1, 2))  # [B, C, H, W]

    xp = np.zeros((B, 2, C, PR, PW), np.float32)
    xp[:, 0, :, 1:65, 1:129] = xt[:, :, 0:64, :]
    xp[:, 1, :, 1:65, 1:129] = xt[:, :, 64:128, :]
    # halo rows between halves
    xp[:, 0, :, 65, 1:129] = xt[:, :, 64, :]
    xp[:, 1, :, 0, 1:129] = xt[:, :, 63, :]
    xp = xp.reshape(B, 2, C, NPAD)

    u = (rand_mask[..., 0] <= 0.5).astype(np.float32).reshape(B, 2, 64, W)
    u16 = np.ascontiguousarray(
        np.broadcast_to(u[:, :, None], (B, 2, C, 64, W))
    ).astype(bf).reshape(B, 2, C, NPIX)

    apad = np.zeros((B, H + 2, PW), np.float32)
    apad[:, 1:129, 1:129] = x[..., 3]
    idx = 4 * np.arange(32)[:, None] + np.arange(6)[None, :]
    astr = apad[:, idx, :].reshape(B, 32, 780)  # [B, strip, 6*130]

    W0id = w0[:, 0::3]
    W0dx = w0[:, 1::3] / 8.0
    W0dy = w0[:, 2::3] / 8.0
    w048 = np.zeros((128, 128), bf)
    blk = np.concatenate([W0id.T, W0dx.T, W0dy.T], axis=0).astype(bf)  # [48, 128]
    w048[0:48, :] = blk
    w048[64:112, :] = blk
    w1lo = np.zeros((128, 32), bf)
    w1lo[:, 0:16] = w1.T.astype(bf)
    w1hi = np.zeros((128, 32), bf)
    w1hi[:, 16:32] = w1.T.astype(bf)

    sel = np.zeros((128, 2048), bf)
    for t in range(16):
        for p in range(128):
            g = p // 16
            sel[16 * g + t, 128 * t + p] = 1.0

    in_maps = []
    for ci in range(N_CORES):
        sl = slice(IMGS * ci, IMGS * (ci + 1))
        m = {
            "xpad": np.ascontiguousarray(xp[sl]).reshape(128, NPAD),
            "u16": np.ascontiguousarray(u16[sl]).reshape(128, NPIX),
            "astrip": np.ascontiguousarray(astr[sl]).reshape(128, 780),
            "sel": sel,
            "w048": w048,
            "w1lo": w1lo,
            "w1hi": w1hi,
        }
        in_maps.append(m)
    return in_maps


def host_post(results):
    out = np.empty((B, H, W, C), np.float32)
    for ci in range(N_CORES):
        o = results[ci]["out"].reshape(IMGS, 2, C, 64, W)
        out[IMGS * ci : IMGS * (ci + 1)] = o.transpose(0, 1, 3, 4, 2).reshape(
            IMGS, H, W, C
        )
    return out


_CACHE = {}


def kernel(x, w0, w1, rand_mask, _trace=False):
    x = np.asarray(x, np.float32)
    w0 = np.asarray(w0, np.float32)
    w1 = np.asarray(w1, np.float32)
    rand_mask = np.asarray(rand_mask, np.float32)

    if "nc" not in _CACHE:
        _CACHE["nc"] = build_program()
    nc = _CACHE["nc"]

    in_maps = host_prep(x, w0, w1, rand_mask)
    res = bass_utils.run_bass_kernel_spmd(
        nc, in_maps, core_ids=list(range(N_CORES)), trace=_trace
    )
    _CACHE["last_result"] = res
    return host_post(res.results)


# revision 14
# speedup vs baseline: 1.3050x; 1.0606x over previous
"""Trainium2 Bass kernel for nn_CAModel (neural cellular automaton step).

Strategy (pure data parallel, B=32 -> 4 images per core x 8 cores):
- Host pre-transposes to channel-major padded layout; device partition p =
  (img_local, half, channel) = 4*2*16 = 128.  All spatial shifts become
  free-dim offsets (row pitch 130, zero ring).
- Depthwise sobel conv as separable shifted adds on VectorE in bf16,
  band-local temps (16 interior rows per band), fused (2x + t) via
  scalar_tensor_tensor.
- Conv dx/dy outputs land in pitch-128 band buffers and are repacked by
  plain-slice SBUF DMAs (16 long descriptors each) into 48-feature
  K-strips: y48 tiles, partitions 64*sg + (16*feat + chan), free =
  (quad, row, col).  The id feature arrives pre-packed from DRAM (host
  prep) via real async DMA.  fc0 is then ONE K=48 matmul per 512-px
  window per group (3x fewer PE rows than feature-accumulation).
- fc1: K=128 M=32 matmuls with zero-padded w1lo/w1hi writing the two
  16-partition halves of each 32-strip (accumulated pair, junk-free).
- MLP chunks pair two groups: hp/rh are [128,1024] (halved relu count);
  per 512-px window the 8 group matmuls fill all 128 dxp PSUM
  partitions -> full-width residual + fire-mask ops.
- relu PSUM->SBUF mostly on ScalarE (GpSimd can't touch PSUM or do
  tensor_tensor on TRN2); alive-mask maxpool in strip layout on VectorE;
  life broadcast via PE sel-matmul.
"""

import dataclasses
import numpy as np
import ml_dtypes

import concourse.bass as bass
import concourse.tile as tile
from concourse import mybir, bass_utils

F32 = mybir.dt.float32
BF16 = mybir.dt.bfloat16
ALU = mybir.AluOpType
ACTF = mybir.ActivationFunctionType

N_CORES = 8
B, H, W, C = 32, 128, 128, 16
HID = 128
IMGS = B // N_CORES          # 4 images per core
PW = W + 2                   # padded row pitch 130
PR = H // 2 + 2              # padded rows per half 66
NPAD = PR * PW               # 8580
NPIX = (H // 2) * W          # 8192 interior pixels per group
X2G = 128                    # guard elems around x2 free dim
PITCH = NPIX + 2 * X2G
NB = 4                       # conv bands, 16 interior rows each
NW = 16                      # MLP windows, 512 px each
RELU_PATTERN = ("s", "s", "s", "s", "s", "s", "s", "s", "s", "s", "s", "v")


def _split_multiwaits(nc):
    """walrus in this env only supports one sem-wait per instruction."""
    n = 0
    for f in nc.m.functions:
        for bb in f.blocks:
            out = []
            changed = False
            for inst in bb.instructions:
                si = inst.sync_info
                if si is not None and len(si.on_wait) > 1:
                    waits = list(si.on_wait)
                    for k, w in enumerate(waits[:-1]):
                        nop = mybir.InstNoOp(
                            name=f"{inst.name}_ws{k}",
                            sync_info=mybir.SyncInfo(on_wait=[w], on_update=[]),
                            bass_nofuse=True,
                            engine=inst.engine,
                        )
                        nc.register_instruction(nop, overwrite=True)
                        out.append(nop)
                        n += 1
                    inst.sync_info = mybir.SyncInfo(
                        on_wait=[waits[-1]], on_update=list(si.on_update)
                    )
                    changed = True
                out.append(inst)
            if changed:
                bb.instructions[:] = out
    return n


def _mk_ap(ap, offset, dims):
    return dataclasses.replace(ap, offset=offset, ap=[list(d) for d in dims])


def build_program():
    nc = bass.Bass()

    xpad_d = nc.dram_tensor("xpad", [128, NPAD], F32, kind="ExternalInput").ap()
    xid_d = nc.dram_tensor("xid", [32, 32768], BF16, kind="ExternalInput").ap()
    u16_d = nc.dram_tensor("u16", [128, NPIX], BF16, kind="ExternalInput").ap()
    astrip_d = nc.dram_tensor("astrip", [128, 780], F32, kind="ExternalInput").ap()
    w048_d = nc.dram_tensor("w048", [128, 128], BF16, kind="ExternalInput").ap()
    w1lo_d = nc.dram_tensor("w1lo", [128, 32], BF16, kind="ExternalInput").ap()
    w1hi_d = nc.dram_tensor("w1hi", [128, 32], BF16, kind="ExternalInput").ap()
    sel_d = nc.dram_tensor("sel", [128, 2048], BF16, kind="ExternalInput").ap()
    out_d = nc.dram_tensor("out", [128, NPIX], F32, kind="ExternalOutput").ap()

    with tile.TileContext(nc) as tc:
        with tc.tile_pool(name="persist", bufs=1) as pp:
            xpad = pp.tile([128, NPAD], F32, tag="xpad")
            y48 = [
                pp.tile([128, 4 * 2048], BF16, tag=f"y48_{t}", name=f"y48_{t}")
                for t in range(4)
            ]
            u16 = pp.tile([128, NPIX], BF16, tag="u16")
            astrip = pp.tile([128, 780], F32, tag="astrip")
            a2strip = pp.tile([128, 780], F32, tag="a2strip")
            selt = pp.tile([128, 2048], BF16, tag="selt")
            w048t = pp.tile([128, 128], BF16, tag="w048")
            w1t = [pp.tile([128, 32], BF16, tag=f"w1_{i}", name=f"w1_{i}") for i in range(2)]
            prepool = pp.tile([128, 512], F32, tag="prepool")
            postpool = pp.tile([128, 512], F32, tag="postpool")
            pmin = pp.tile([128, 512], F32, tag="pmin")
            lifes = pp.tile([128, 512], BF16, tag="lifes")

            # --- input DMAs ---
            nc.sync.dma_start(out=w048t[:, :], in_=w048_d)
            nc.sync.dma_start(out=w1t[0][:, :], in_=w1lo_d)
            nc.sync.dma_start(out=w1t[1][:, :], in_=w1hi_d)
            CAST_BANDS = [(0, 18), (18, 34), (34, 50), (50, 66)]
            for lo, hi in CAST_BANDS:
                nc.sync.dma_start(
                    out=xpad[:, lo * PW : hi * PW], in_=xpad_d[:, lo * PW : hi * PW]
                )
            # id feature pre-packed on host -> straight into y48 id strips
            for t in range(4):
                for sg in range(2):
                    nc.scalar.dma_start(
                        out=y48[t][64 * sg : 64 * sg + 16, :],
                        in_=xid_d[16 * sg : 16 * sg + 16, t * 8192 : (t + 1) * 8192],
                    )
            nc.scalar.dma_start(out=u16[:, :], in_=u16_d)
            nc.scalar.dma_start(out=astrip[:, :], in_=astrip_d)
            nc.scalar.dma_start(out=selt[:, :], in_=sel_d)
            nc.vector.memset(a2strip[:, :], 0.0)

            def pool3(src_t, dst_t, eng):
                """3x3 maxpool in the 6-row strip layout (4 out rows/strip)."""
                vm = pp.tile([128, 524], F32, tag="vm")
                t1 = pp.tile([128, 524], F32, tag="t1")
                t2 = pp.tile([128, 524], F32, tag="t2")
                eng.tensor_tensor(
                    out=t1[:, 0:520], in0=src_t[:, 0:520], in1=src_t[:, 130:650],
                    op=ALU.max,
                )
                eng.tensor_tensor(
                    out=vm[:, 0:520], in0=t1[:, 0:520], in1=src_t[:, 260:780],
                    op=ALU.max,
                )
                eng.tensor_tensor(
                    out=t2[:, 0:519], in0=vm[:, 0:519], in1=vm[:, 1:520],
                    op=ALU.max,
                )
                vmr = vm[:, 0:520].rearrange("p (r w) -> p r w", w=130)
                t2r = t2[:, 0:520].rearrange("p (r w) -> p r w", w=130)
                dstr = dst_t[:, :].rearrange("p (r w) -> p r w", w=128)
                eng.tensor_tensor(
                    out=dstr[:, 0:4, :], in0=t2r[:, 0:4, 0:128],
                    in1=vmr[:, 0:4, 2:130], op=ALU.max,
                )

            # --- conv (16-row bands, band-local temps) + repack DMAs ---
            with tc.tile_pool(name="convp", bufs=1) as cp:
                xb = cp.tile([128, NPAD], BF16, tag="xb")
                for lo, hi in CAST_BANDS:
                    nc.scalar.activation(
                        out=xb[:, lo * PW : hi * PW],
                        in_=xpad[:, lo * PW : hi * PW],
                        func=ACTF.Copy,
                    )
                xb3 = xb[:, :].rearrange("p (r w) -> p r w", w=PW)
                dma_engs = (nc.sync, nc.gpsimd)
                dma_i = 0
                for b in range(NB):
                    r0 = 16 * b  # temp base padded row; interior r0+1..r0+17
                    tb = cp.tile([128, 18 * PW], BF16, tag="tb")
                    th = cp.tile([128, 18 * PW], BF16, tag="th")
                    ta = cp.tile([128, 16 * PW], BF16, tag="ta")
                    tv = cp.tile([128, 16 * PW], BF16, tag="tv")
                    ydxb = cp.tile([128, 16 * 128], BF16, tag="ydx", bufs=2)
                    ydyb = cp.tile([128, 16 * 128], BF16, tag="ydy", bufs=2)
                    tb3 = tb[:, :].rearrange("p (r w) -> p r w", w=PW)
                    th3 = th[:, :].rearrange("p (r w) -> p r w", w=PW)
                    ta3 = ta[:, :].rearrange("p (r w) -> p r w", w=PW)
                    tv3 = tv[:, :].rearrange("p (r w) -> p r w", w=PW)
                    ydx3 = ydxb[:, :].rearrange("p (r w) -> p r w", w=128)
                    ydy3 = ydyb[:, :].rearrange("p (r w) -> p r w", w=128)
                    nc.vector.tensor_tensor(
                        out=tb3[:, :, 1:129], in0=xb3[:, r0 : r0 + 18, 0:128],
                        in1=xb3[:, r0 : r0 + 18, 2:130], op=ALU.add,
                    )
                    nc.vector.scalar_tensor_tensor(
                        out=th3[:, :, 1:129], in0=xb3[:, r0 : r0 + 18, 1:129],
                        scalar=2.0, in1=tb3[:, :, 1:129],
                        op0=ALU.mult, op1=ALU.add,
                    )
                    nc.vector.tensor_tensor(
                        out=ta3[:, :, :], in0=xb3[:, r0 : r0 + 16, :],
                        in1=xb3[:, r0 + 2 : r0 + 18, :], op=ALU.add,
                    )
                    nc.vector.scalar_tensor_tensor(
                        out=tv3[:, :, :], in0=xb3[:, r0 + 1 : r0 + 17, :],
                        scalar=2.0, in1=ta3[:, :, :],
                        op0=ALU.mult, op1=ALU.add,
                    )
                    nc.vector.tensor_tensor(
                        out=ydx3[:, :, :], in0=tv3[:, :, 2:130],
                        in1=tv3[:, :, 0:128], op=ALU.subtract,
                    )
                    nc.vector.tensor_tensor(
                        out=ydy3[:, :, :], in0=th3[:, 2:18, 1:129],
                        in1=th3[:, 0:16, 1:129], op=ALU.subtract,
                    )
                    # repack: band b == y48 tile b; plain-slice SBUF DMAs
                    for sg in range(2):
                        for ft, src_t in ((1, ydxb), (2, ydyb)):
                            for q in range(4):
                                g = 4 * sg + q
                                dma_engs[dma_i % 2].dma_start(
                                    out=y48[b][
                                        64 * sg + 16 * ft : 64 * sg + 16 * ft + 16,
                                        q * 2048 : (q + 1) * 2048,
                                    ],
                                    in_=src_t[16 * g : 16 * g + 16, :],
                                )
                                dma_i += 1

            # pre-life maxpool
            pool3(astrip, prepool, nc.vector)

            # --- MLP + residual: 1024-px chunks (2 groups), lag-2 pipeline ---
            lp_cm = tc.tile_pool(name="late", bufs=1)
            lp = lp_cm.__enter__()
            x2 = lp.tile([128, NPIX + 2 * X2G], F32, tag="x2")
            nc.vector.memset(x2[:, 0:X2G], 0.0)
            nc.vector.memset(x2[:, X2G + NPIX : NPIX + 2 * X2G], 0.0)
            x2r = x2[:, X2G : X2G + NPIX].rearrange("p (r w) -> p r w", w=W)
            xintr = xpad[:, :].rearrange("p (r w) -> p r w", w=PW)

            with (
                tc.tile_pool(name="mlp", bufs=1) as mp,
                tc.tile_pool(name="psum", bufs=1, space="PSUM") as psp,
            ):
                stages = [(w, p) for w in range(NW) for p in range(4)]
                hp_t = {}
                rh_t = {}
                dxp_t = {}
                relu_i = 0

                def emit_fc0(i):
                    w, p = stages[i]
                    sg = p // 2
                    t, lw = w // 4, w % 4
                    hp = psp.tile([128, 1024], F32, tag="hp", bufs=3)
                    hp_t[i] = hp
                    for half in range(2):
                        q = (2 * p + half) % 4
                        nc.tensor.matmul(
                            hp[:, half * 512 : half * 512 + 512],
                            w048t[64 * sg : 64 * sg + 48, :],
                            y48[t][
                                64 * sg : 64 * sg + 48,
                                q * 2048 + lw * 512 : q * 2048 + lw * 512 + 512,
                            ],
                            start=True, stop=True,
                            tile_position=(64 * sg, 0),
                        )

                def emit_relu(i):
                    nonlocal relu_i
                    rh = mp.tile([128, 1024], BF16, tag="rh", bufs=3)
                    rh_t[i] = rh
                    kind = RELU_PATTERN[relu_i % len(RELU_PATTERN)]
                    relu_i += 1
                    hp = hp_t.pop(i)
                    if kind == "s":
                        nc.scalar.activation(out=rh[:, :], in_=hp[:, :], func=ACTF.Relu)
                    else:
                        nc.vector.tensor_scalar_max(rh[:, :], hp[:, :], 0.0)

                def emit_fc1(i):
                    w, p = stages[i]
                    if p == 0:
                        dxp_t[w] = psp.tile(
                            [128, 512], F32, tag="dxp", bufs=2, name="dxp"
                        )
                    dxp = dxp_t[w]
                    rh = rh_t.pop(i)
                    for half in range(2):
                        nc.tensor.matmul(
                            dxp[32 * p : 32 * p + 32, :],
                            w1t[half][:, :],
                            rh[:, half * 512 : half * 512 + 512],
                            start=(half == 0), stop=(half == 1),
                            tile_position=(0, 32 * p),
                        )

                def emit_epilogue(w):
                    # mask-mult drains dx PSUM->SBUF on Vector, residual add V
                    dxp = dxp_t.pop(w)
                    st = lp.tile([128, 512], F32, tag="st", bufs=2)
                    nc.vector.tensor_tensor(
                        out=st[:, :], in0=dxp[:, :],
                        in1=u16[:, w * 512 : (w + 1) * 512], op=ALU.mult,
                    )
                    st3 = st[:, :].rearrange("p (r w) -> p r w", w=W)
                    nc.vector.tensor_tensor(
                        out=x2r[:, 4 * w : 4 * w + 4, :],
                        in0=xintr[:, 1 + 4 * w : 5 + 4 * w, 1:129],
                        in1=st3,
                        op=ALU.add,
                    )

                LAG = 2
                for i in range(len(stages) + LAG):
                    if i < len(stages):
                        emit_fc0(i)
                    j = i - LAG
                    if j >= 0:
                        emit_relu(j)
                        emit_fc1(j)
                        wj, pj = stages[j]
                        if pj == 3:
                            emit_epilogue(wj)

            # --- alive masks ---
            x2ap = x2[:, :]
            a2ap = a2strip[:, :]
            for r in range(6):
                src = _mk_ap(
                    x2ap, 3 * PITCH + 128 * r,
                    [[16 * PITCH, 8], [512, 16], [1, 128]],
                )
                dst = _mk_ap(a2ap, 130 * r + 1, [[780, 128], [1, 128]])
                eng = nc.sync if r % 2 == 0 else nc.scalar
                eng.dma_start(out=dst, in_=src)
            nc.sync.dma_start(
                out=_mk_ap(a2ap, 15 * 780 + 5 * 130 + 1, [[32 * 780, 4], [1, 128]]),
                in_=_mk_ap(x2ap, 19 * PITCH + X2G, [[32 * PITCH, 4], [1, 128]]),
            )
            nc.scalar.dma_start(
                out=_mk_ap(a2ap, 16 * 780 + 1, [[32 * 780, 4], [1, 128]]),
                in_=_mk_ap(
                    x2ap, 3 * PITCH + X2G + 63 * 128, [[32 * PITCH, 4], [1, 128]]
                ),
            )
            pool3(a2strip, postpool, nc.vector)
            nc.vector.tensor_tensor(
                out=pmin[:, :], in0=prepool[:, :], in1=postpool[:, :], op=ALU.min
            )
            nc.vector.tensor_scalar(
                out=lifes[:, :], in0=pmin[:, :], scalar1=0.1, scalar2=None,
                op0=ALU.is_gt,
            )

            # --- final mask multiply + store ---
            with tc.tile_pool(name="psum2", bufs=1, space="PSUM") as psp2:
                for k in range(4):
                    lps = psp2.tile([128, 2048], F32, tag="lps", name=f"lps{k}", bufs=2)
                    for tl in range(4):
                        t = 4 * k + tl
                        nc.tensor.matmul(
                            lps[:, 512 * tl : 512 * tl + 512],
                            selt[:, 128 * t : 128 * t + 128],
                            lifes[:, 0:512],
                            start=True,
                            stop=True,
                        )
                    ot = lp.tile([128, 2048], F32, tag="ot", name=f"ot{k}", bufs=2)
                    nc.vector.tensor_tensor(
                        out=ot[:, :],
                        in0=x2[:, X2G + 2048 * k : X2G + 2048 * (k + 1)],
                        in1=lps[:, :],
                        op=ALU.mult,
                    )
                    deng = nc.sync if k % 2 == 0 else nc.scalar
                    deng.dma_start(
                        out=out_d[:, 2048 * k : 2048 * (k + 1)], in_=ot[:, :]
                    )

            lp_cm.__exit__(None, None, None)

    _split_multiwaits(nc)
    return nc


def host_prep(x, w0, w1, rand_mask):
    bf = ml_dtypes.bfloat16
    xt = np.ascontiguousarray(x.transpose(0, 3, 1, 2))  # [B, C, H, W]

    xp = np.zeros((B, 2, C, PR, PW), np.float32)
    xp[:, 0, :, 1:65, 1:129] = xt[:, :, 0:64, :]
    xp[:, 1, :, 1:65, 1:129] = xt[:, :, 64:128, :]
    xp[:, 0, :, 65, 1:129] = xt[:, :, 64, :]
    xp[:, 1, :, 0, 1:129] = xt[:, :, 63, :]
    xp = xp.reshape(B, 2, C, NPAD)

    u = (rand_mask[..., 0] <= 0.5).astype(np.float32).reshape(B, 2, 64, W)
    u16 = np.ascontiguousarray(
        np.broadcast_to(u[:, :, None], (B, 2, C, 64, W))
    ).astype(bf).reshape(B, 2, C, NPIX)

    apad = np.zeros((B, H + 2, PW), np.float32)
    apad[:, 1:129, 1:129] = x[..., 3]
    idx = 4 * np.arange(32)[:, None] + np.arange(6)[None, :]
    astr = apad[:, idx, :].reshape(B, 32, 780)  # [B, strip, 6*130]

    W0id = w0[:, 0::3]
    W0dx = w0[:, 1::3] / 8.0
    W0dy = w0[:, 2::3] / 8.0
    w048 = np.zeros((128, 128), bf)
    blk = np.concatenate([W0id.T, W0dx.T, W0dy.T], axis=0).astype(bf)  # [48, 128]
    w048[0:48, :] = blk
    w048[64:112, :] = blk
    w1lo = np.zeros((128, 32), bf)
    w1lo[:, 0:16] = w1.T.astype(bf)
    w1hi = np.zeros((128, 32), bf)
    w1hi[:, 16:32] = w1.T.astype(bf)

    sel = np.zeros((128, 2048), bf)
    for t in range(16):
        for p in range(128):
            g = p // 16
            sel[16 * g + t, 128 * t + p] = 1.0

    # id feature pre-packed: xid[16*sg + c, t*8192 + q*2048 + lr*128 + col]
    # = x[(core img) 2*sg + q//2, half q%2, chan c, pixel row 16t+lr, col]
    xtb = xt.astype(bf)  # [B, C, H, W]
    in_maps = []
    for ci in range(N_CORES):
        sl = slice(IMGS * ci, IMGS * (ci + 1))
        xtc = xtb[sl]  # [4, C, H, W]
        xid = np.zeros((32, 4, 4, 16, 128), bf)  # [sgc, t, q, lr, col]
        for sg in range(2):
            for q in range(4):
                img, half = 2 * sg + q // 2, q % 2
                grid = xtc[img, :, 64 * half : 64 * half + 64, :]  # [C, 64, 128]
                xid[16 * sg : 16 * sg + 16, :, q] = grid.reshape(C, 4, 16, 128)
        m = {
            "xpad": np.ascontiguousarray(xp[sl]).reshape(128, NPAD),
            "xid": xid.reshape(32, 32768),
            "u16": np.ascontiguousarray(u16[sl]).reshape(128, NPIX),
            "astrip": np.ascontiguousarray(astr[sl]).reshape(128, 780),
            "sel": sel,
            "w048": w048,
            "w1lo": w1lo,
            "w1hi": w1hi,
        }
        in_maps.append(m)
    return in_maps


def host_post(results):
    out = np.empty((B, H, W, C), np.float32)
    for ci in range(N_CORES):
        o = results[ci]["out"].reshape(IMGS, 2, C, 64, W)
        out[IMGS * ci : IMGS * (ci + 1)] = o.transpose(0, 1, 3, 4, 2).reshape(
            IMGS, H, W, C
        )
    return out


_CACHE = {}


def kernel(x, w0, w1, rand_mask, _trace=False):
    x = np.asarray(x, np.float32)
    w0 = np.asarray(w0, np.float32)
    w1 = np.asarray(w1, np.float32)
    rand_mask = np.asarray(rand_mask, np.float32)

    if "nc" not in _CACHE:
        _CACHE["nc"] = build_program()
    nc = _CACHE["nc"]

    in_maps = host_prep(x, w0, w1, rand_mask)
    res = bass_utils.run_bass_kernel_spmd(
        nc, in_maps, core_ids=list(range(N_CORES)), trace=_trace
    )
    _CACHE["last_result"] = res
    return host_post(res.results)
